# revision 16
# baseline (speedup 1.0000x reference)
"""Multi-head causal attention (B=2, S=2048, D=2048, H=16) on 8 Trainium2 NeuronCores.

Sharding: tensor-parallel over heads — 2 heads per core. Each core computes
QKV projections for its heads (full token range), causal attention, and a
partial output projection through its slice of W_o. The host sums the 8
partial outputs (the TP all-reduce) to produce the full result.

Per-core device pipeline (all matmuls in float32r: full-rate fp32 on the PE):
  1. x [4096, 2048] is transposed on-chip (PE transpose, 128x128 tiles) to
     xT tiles [d, tok] because the PE contracts over the partition dim.
  2. Qt/Kt [dk, tok] and V [tok, dk] per (batch, head) via matmuls vs
     host-pre-transposed weight slices (1/sqrt(dk) folded into W_q on host).
  3. Scores transposed: sT[k, q] = Kt_tile^T @ Qt  -> PSUM, causal mask added
     on diagonal tiles (staircase mask from host), exp on the scalar engine.
  4. l[q] = sum_k exp(sT) via ones-vector matmul; O^T[dk, q] = V_tile^T? no:
     lhsT=V[k,dk], rhs=exp_sT[k,q] accumulated over k tiles.  O^T normalized
     by broadcasting 1/l across partitions with a rank-1 ones matmul.
  5. y_partial[tok, o] = sum_heads O^T_h[:, tok]^T @ W_oT_h[:, o].
"""

import os
import sys

for _p in ("/opt/trn_rl_repo", "/root/.axon_site/_ro/trn_rl_repo"):
    if os.path.isdir(_p) and _p not in sys.path:
        sys.path.insert(0, _p)
        break

import numpy as np

import concourse.bass as bass
import concourse.mybir as mybir
import concourse.tile as tile
from concourse import bacc
from concourse.bass_utils import run_bass_kernel_spmd

B, S, D, H = 2, 2048, 2048, 16
DK = D // H            # 128
N_CORES = 8
HPC = H // N_CORES     # heads per core = 2
NTOK = B * S           # 4096
MASKV = -1e10

FP = mybir.dt.float32
FPR = mybir.dt.float32r

# token chunking
CHUNK = 512            # tokens per QKV chunk
NCHUNK = S // CHUNK    # 4 per batch
DT = D // 128          # 16 d-tiles
QW = 512               # query tile width in attention
NQ = S // QW           # 4
KT = S // 128          # 16 k tiles per batch


def _build_program(rep: int = 1, phases=("qkv", "attn", "wo")):
    nc = bacc.Bacc("TRN2", target_bir_lowering=False, debug=False,
                   num_devices=N_CORES)

    xT = nc.dram_tensor("xT", [D, NTOK], FPR, kind="ExternalInput").ap()
    wqT = nc.dram_tensor("wqT", [D, HPC * DK], FPR, kind="ExternalInput").ap()
    wkT = nc.dram_tensor("wkT", [D, HPC * DK], FPR, kind="ExternalInput").ap()
    wvT = nc.dram_tensor("wvT", [D, HPC * DK], FPR, kind="ExternalInput").ap()
    woT = nc.dram_tensor("woT", [HPC * DK, D], FPR, kind="ExternalInput").ap()
    mask = nc.dram_tensor("mask", [128, 896], FP, kind="ExternalInput").ap()
    ones = nc.dram_tensor("ones", [128, 128], FPR, kind="ExternalInput").ap()
    y = nc.dram_tensor("y", [NTOK, D], mybir.dt.float16,
                       kind="ExternalOutput").ap()

    EXP = mybir.ActivationFunctionType.Exp

    with tile.TileContext(nc) as tc, nc.allow_low_precision(
        reason="float32r is bit-identical to float32"
    ):
        with (
            tc.tile_pool(name="const", bufs=1) as constp,
            tc.tile_pool(name="w", bufs=1) as wp,
            tc.tile_pool(name="xT", bufs=20) as xTp,
            tc.tile_pool(name="qk", bufs=4) as qkp,
            tc.tile_pool(name="v", bufs=32) as vp,
            tc.tile_pool(name="expp", bufs=5) as expp,
            tc.tile_pool(name="ot", bufs=2) as otp,
            tc.tile_pool(name="small", bufs=4) as smallp,
            tc.tile_pool(name="wo", bufs=8) as wop,
            tc.tile_pool(name="yout", bufs=2) as yp,
            tc.tile_pool(name="ps", bufs=8, space="PSUM") as psp,
        ):
            # ---- constants & weights (loaded once) ----
            mask_sb = constp.tile([128, 896], FP)
            nc.sync.dma_start(mask_sb[:], mask[:])
            ones_sb = constp.tile([128, 128], FPR)
            nc.sync.dma_start(ones_sb[:], ones[:])

            w_sb = {}
            for name, src in (("q", wqT), ("k", wkT), ("v", wvT)):
                t = wp.tile([128, DT, HPC * DK], FPR, tag=f"w{name}")
                nc.gpsimd.dma_start(
                    t[:], src.rearrange("(t p) m -> p t m", p=128))
                w_sb[name] = t

            import contextlib
            loop_ctx = tc.For_i(0, rep, 1) if rep > 1 else contextlib.nullcontext()
            with loop_ctx:
                _emit_body(nc, tc, locals(), phases)
    nc.compile()
    return nc


def _emit_body(nc, tc, env, phases=("qkv", "attn", "wo")):
    xTd = env["xT"]; y = env["y"]
    mask_sb = env["mask_sb"]; ones_sb = env["ones_sb"]
    w_sb = env["w_sb"]; woT = env["woT"]
    xTp = env["xTp"]; qkp = env["qkp"]; vp = env["vp"]
    expp = env["expp"]; otp = env["otp"]; smallp = env["smallp"]
    wop = env["wop"]; yp = env["yp"]; psp = env["psp"]
    EXP = env["EXP"]
    if True:
        if True:
            for b in range(B):
                base = b * S
                # ================= QKV phase =================
                qt = [qkp.tile([128, S], FPR, tag="qk", name=f"qt_{b}_{i}") for i in range(HPC)]
                kt = [qkp.tile([128, S], FPR, tag="qk", name=f"kt_{b}_{i}") for i in range(HPC)]
                vt = [[None] * KT for _ in range(HPC)]
                for c in range(NCHUNK):
                    xT = [xTp.tile([128, CHUNK], FPR, tag="xT", name=f"xT_{b}_{c}_{i}")
                          for i in range(DT)]
                    tok0 = base + c * CHUNK
                    for t in range(DT):
                        eng = nc.sync if t % 2 == 0 else nc.scalar
                        eng.dma_start(
                            xT[t][:], xTd[t * 128:(t + 1) * 128,
                                          tok0:tok0 + CHUNK])
                    # 8 accumulation chains advance together per d-tile so
                    # each xT[t] is released after 8 consecutive matmuls
                    # (earlier DMA prefetch of the next chunk).
                    chains = ((qt[0], "q", 0), (qt[1], "q", 1),
                              (kt[0], "k", 0), (kt[1], "k", 1))
                    ps_qk = [psp.tile([128, CHUNK], FP, tag="ps",
                                      name=f"psqk{i}") for i in range(4)]
                    ps_v = [psp.tile([128, CHUNK], FP, tag="ps",
                                     name=f"psv{s}") for s in range(4)]
                    for t in range(DT):
                        for i, (dst, wname, blk) in enumerate(chains):
                            nc.tensor.matmul(
                                ps_qk[i][:],
                                w_sb[wname][:, t, blk * 128:(blk + 1) * 128],
                                xT[t][:],
                                start=(t == 0), stop=(t == DT - 1))
                        for s in range(4):
                            nc.tensor.matmul(
                                ps_v[s][:, 0:HPC * DK],
                                xT[t][:, s * 128:(s + 1) * 128],
                                w_sb["v"][:, t, :],
                                start=(t == 0), stop=(t == DT - 1))
                    for i, (dst, wname, blk) in enumerate(chains):
                        nc.vector.tensor_copy(
                            dst[:, c * CHUNK:(c + 1) * CHUNK], ps_qk[i][:])
                    for s in range(4):
                        j = c * 4 + s
                        for h in range(HPC):
                            vtile = vp.tile([128, DK], FPR, tag="v")
                            nc.vector.tensor_copy(
                                vtile[:], ps_v[s][:, h * DK:(h + 1) * DK])
                            vt[h][j] = vtile

                if "attn" not in phases:
                    continue
                # ================= attention phase =================
                ot_sb = []

                def normalize(pend):
                    otps_p, lps_p, ot_p, qi_p = pend
                    rsb = smallp.tile([1, QW], FPR, tag="recip", name="rsb")
                    nc.vector.reciprocal(rsb[:], lps_p[:])
                    rbps = psp.tile([128, QW], FP, tag="ps", name="rbps")
                    nc.tensor.matmul(rbps[:], ones_sb[0:1, :], rsb[:],
                                     start=True, stop=True)
                    rb_sb = smallp.tile([128, QW], FP, tag="rb", name="rb_sb")
                    nc.scalar.copy(rb_sb[:], rbps[:])
                    nc.vector.tensor_mul(
                        ot_p[:, qi_p * QW:(qi_p + 1) * QW], otps_p[:], rb_sb[:])

                pending = None
                PRO = 3  # sT/exp emission lookahead over l/AV consumers
                for h in range(HPC):
                    ot = otp.tile([128, S], FPR, tag="ot", name=f"ot_{b}_{h}")
                    for qi in range(NQ):
                        otps = psp.tile([128, QW], FP, tag="ps", name="otps")
                        lps = psp.tile([1, QW], FP, tag="ps", name="lps")
                        nk = 4 * qi + 4
                        ets = {}

                        def emit_st_exp(j):
                            sps = psp.tile([128, QW], FP, tag="ps", name="sps")
                            nc.tensor.matmul(
                                sps[:],
                                kt[h][:, j * 128:(j + 1) * 128],
                                qt[h][:, qi * QW:(qi + 1) * QW],
                                start=True, stop=True)
                            if j >= 4 * qi:  # diagonal 128-tile: causal mask
                                r = j - 4 * qi
                                nc.vector.tensor_add(
                                    sps[:], sps[:],
                                    mask_sb[:, 384 - 128 * r:896 - 128 * r])
                            et = expp.tile([128, QW], FPR, tag="exp", name="et")
                            nc.scalar.activation(et[:], sps[:], EXP)
                            return et

                        for j in range(nk + PRO):
                            if j < nk:
                                ets[j] = emit_st_exp(j)
                            jj = j - PRO
                            if jj < 0:
                                continue
                            et = ets.pop(jj)
                            nc.tensor.matmul(
                                lps[:], ones_sb[:, 0:1], et[:],
                                start=(jj == 0), stop=(jj == nk - 1),
                                skip_group_check=True)
                            nc.tensor.matmul(
                                otps[:], vt[h][jj][:], et[:],
                                start=(jj == 0), stop=(jj == nk - 1),
                                skip_group_check=True)
                            if jj == 1 and pending is not None:
                                normalize(pending)
                                pending = None
                        pending = (otps, lps, ot, qi)
                    ot_sb.append(ot)
                if pending is not None:
                    normalize(pending)
                    pending = None

                if "wo" not in phases:
                    continue
                # ================= output projection =================
                wo_sl = {}
                for oc in range(NQ):
                    for h in range(HPC):
                        wt = wop.tile([128, QW], FPR, tag="wo",
                                      name=f"wo_{oc}_{h}")
                        eng = nc.sync if (oc + h) % 2 == 0 else nc.scalar
                        eng.dma_start(
                            wt[:],
                            woT[h * DK:(h + 1) * DK,
                                oc * QW:(oc + 1) * QW])
                        wo_sl[(oc, h)] = wt
                for tt in range(KT):
                    ysb = yp.tile([128, D], mybir.dt.float16, tag="y",
                                  name="ysb")
                    for oc in range(NQ):
                        yps = psp.tile([128, QW], FP, tag="ps", name="yps")
                        for h in range(HPC):
                            nc.tensor.matmul(
                                yps[:],
                                ot_sb[h][:, tt * 128:(tt + 1) * 128],
                                wo_sl[(oc, h)][:],
                                start=(h == 0), stop=(h == HPC - 1))
                        nc.vector.tensor_copy(
                            ysb[:, oc * QW:(oc + 1) * QW], yps[:])
                    eng = nc.sync if tt % 2 == 0 else nc.scalar
                    eng.dma_start(
                        y[base + tt * 128:base + (tt + 1) * 128, :], ysb[:])


_NC_CACHE = None


def _get_program():
    global _NC_CACHE
    if _NC_CACHE is None:
        _NC_CACHE = _build_program()
    return _NC_CACHE


def _host_inputs(x, W_qkv, W_o):
    """Build the per-core input maps (host-side sharding)."""
    xT2d = np.ascontiguousarray(np.asarray(x, np.float32).reshape(NTOK, D).T)
    W_qkv = np.asarray(W_qkv, np.float32)
    W_o = np.asarray(W_o, np.float32)
    scale = np.float32(1.0 / np.sqrt(DK))

    kk = np.arange(128)[:, None]
    cc = np.arange(896)[None, :]
    maskm = np.where(kk <= cc - 384, 0.0, MASKV).astype(np.float32)
    onesm = np.ones((128, 128), np.float32)

    in_maps = []
    for c in range(N_CORES):
        r = slice(c * HPC * DK, (c + 1) * HPC * DK)
        wq = W_qkv[0 * D:1 * D][r] * scale
        wk = W_qkv[1 * D:2 * D][r]
        wv = W_qkv[2 * D:3 * D][r]
        in_maps.append({
            "xT": xT2d,
            "wqT": np.ascontiguousarray(wq.T),
            "wkT": np.ascontiguousarray(wk.T),
            "wvT": np.ascontiguousarray(wv.T),
            "woT": np.ascontiguousarray(W_o[:, r].T),
            "mask": maskm,
            "ones": onesm,
        })
    return in_maps


def kernel(x, W_qkv, W_o):
    nc = _get_program()
    in_maps = _host_inputs(x, W_qkv, W_o)
    res = run_bass_kernel_spmd(nc, in_maps, core_ids=list(range(N_CORES)))
    acc = np.zeros((NTOK, D), np.float32)
    for i in range(N_CORES):
        acc += res.results[i]["y"].astype(np.float32)
    return acc.reshape(B, S, D)
